# revision 27
# baseline (speedup 1.0000x reference)
"""Trainium2 Bass kernel for AttentionReadoutAtom (global-softmax segment reduce).

Math:  scores = x @ w + b ; attn = softmax(scores over all N) ;
       out[s] = sum_{i: label_i = s} attn_i * x_i          -> [50000, 128]

Softmax is shift/scale invariant: exp(score) without max-subtraction is safe
here (scores ~ N(0,1)), and the bias b cancels between numerator and
denominator.  Using xw = x * w (host-side sharding layout prep):

    out[s, d] = sum_{i in s} e_i * xw_i[d] / (w[d] * Z),   Z = sum_i e_i

Sharding (host, inside kernel()):
  * Sort rows by segment label; greedily pack whole segments into blocks of
    1024 rows (8 tiles of 128 rows) covering <= 128 distinct segments each;
    pad each block to 1024 rows with zero rows.  Every segment lives in
    exactly one block -> no cross-core combination of outputs is needed;
    the only global quantity is the softmax denominator Z, reduced on the
    host from the per-row e values (the hint's denominator all-reduce).
  * Blocks are dealt contiguously to 8 cores, padded to equal count B;
    blocks are processed in chunks of 4 (one 8KB-per-partition DMA each).
  * xw ships bf16, pre-arranged so every DMA row is contiguous.
  * idx[p, b*8+t] = t*128 + (label rel. to block), or -1 for pad rows,
    int16 — drives the device-side one-hot build.

Device per chunk ch (4 blocks; Tile framework pipelines chunks):
  * score[p, bt] = sum_d xw[p, bt*128+d]    — ONE DVE tensor_reduce
    (axis=X over the [P, 32, 128] view) per chunk.
  * e = exp(score)                          — ONE ScalarE ACTIVATE [P, 32].
  * Me[p, t*128+s] = e[p,t] if idx matches  — GPSIMD local_scatter per
    block builds the one-hot-times-e matrix for 8 tiles in one op (dst
    zeroed by the op; negative pad indices are skipped).  This moves the
    former DVE/ScalarE per-tile one-hot bottleneck onto the otherwise
    idle GPSIMD engine.
  * psum[s, d] += Me_t^T @ xw_t             — TensorE, 8 matmuls/block
    accumulating in PSUM.
  * evict psum -> SBUF (ScalarE Copy) -> DRAM; e chunk -> DRAM for the
    host-side Z reduction.

Host epilogue: scatter per-block rows to the full [50000, 128] output and
apply the scalar normalization out / (w[d] * Z).
"""

import numpy as np
import ml_dtypes

# ---------------------------------------------------------------- constants
N = 500000
D = 128
NUM_SEGMENTS = 50000
N_CORES = 8
P = 128
TPB = 8                   # row tiles per block
ROWS_PER_BLOCK = TPB * P  # 1024
MAX_SEGS_PER_BLOCK = 128
CHUNK_BLOCKS = 4          # blocks per chunk

_COMPILED = {}


# ---------------------------------------------------------------- device code
def _build_kernel(B):
    import concourse.bacc as bacc
    import concourse.mybir as mybir
    from concourse.tile import TileContext

    f32 = mybir.dt.float32
    bf16 = mybir.dt.bfloat16
    i16 = mybir.dt.int16
    Alu = mybir.AluOpType
    Act = mybir.ActivationFunctionType
    Ax = mybir.AxisListType

    W = TPB * P                      # 1024 columns per block
    NCHUNK = (B + CHUNK_BLOCKS - 1) // CHUNK_BLOCKS
    CC = CHUNK_BLOCKS * TPB          # score/e columns per full chunk

    nc = bacc.Bacc("TRN2", target_bir_lowering=False, debug=False,
                   num_devices=N_CORES)

    xw_d = nc.dram_tensor("xw", [B, P, W], bf16, kind="ExternalInput")
    idx_d = nc.dram_tensor("idx", [P, B * TPB], i16, kind="ExternalInput")
    out_d = nc.dram_tensor("out", [B, P, P], bf16, kind="ExternalOutput")
    z_d = nc.dram_tensor("zpart", [P, B * TPB], bf16, kind="ExternalOutput")

    with TileContext(nc) as tc:
        with tc.tile_pool(name="const", bufs=1) as cpool, \
             tc.tile_pool(name="xwp", bufs=6) as xwp, \
             tc.tile_pool(name="hp", bufs=12) as hp, \
             tc.tile_pool(name="scp", bufs=6) as scp, \
             tc.tile_pool(name="mep", bufs=12) as mep, \
             tc.tile_pool(name="psum", bufs=8, space="PSUM") as psp:

            idx_t = cpool.tile([P, B * TPB], i16)
            zbuf = cpool.tile([P, B * TPB], bf16)
            obuf = cpool.tile([P, B * P], bf16)
            flushed = 0
            z_flushed = 0

            def issue_flush(pend, pend_z=None):
                # issued from the GPSIMD queue one chunk late: that queue
                # paces the kernel, so by the time it reaches this DMA the
                # covered evicts are long done and the wait is free; the
                # only cost is the ~0.65us issue slot on the pacer.  The
                # sync queue (xw prefetch) and ScalarE queue (exp feeding
                # the scatters) are never blocked behind an evict wait.
                if pend is not None:
                    f0, f1 = pend
                    nc.gpsimd.dma_start(
                        out_d.ap()[f0:f1, :, :].rearrange("b p q -> p b q"),
                        obuf[:].rearrange("p (b q) -> p b q", q=P)
                        [:, f0:f1, :])
                if pend_z is not None:
                    z0, z1 = pend_z
                    nc.gpsimd.dma_start(
                        z_d.ap()[:, z0 * TPB:z1 * TPB],
                        zbuf[:, z0 * TPB:z1 * TPB])

            pend = pend_z = None
            for ch in range(NCHUNK):
                b0 = ch * CHUNK_BLOCKS
                nb = min(CHUNK_BLOCKS, B - b0)
                nt = nb * TPB

                issue_flush(pend, pend_z)
                pend = pend_z = None

                xw_c = xwp.tile([P, CHUNK_BLOCKS * W], bf16, tag="xw")
                h_c = hp.tile([P, CC * 64], bf16, tag="h")
                h2_c = hp.tile([P, CC * 32], bf16, tag="h2")
                sc_c = scp.tile([P, CC], f32, tag="sc")
                x3 = xw_c[:].rearrange("p (b w) -> p b w", w=W)
                x4 = xw_c[:].rearrange("p (t d) -> p t d", d=P)
                h4 = h_c[:].rearrange("p (t j) -> p t j", j=64)
                h24 = h2_c[:].rearrange("p (t j) -> p t j", j=32)
                h3 = h2_c[:].rearrange("p (t j) -> p t j", j=32)
                # chunk 0 is processed in 2-block pieces so the first
                # scatter starts sooner; later chunks use one piece
                PZ = 2 if ch == 0 else CHUNK_BLOCKS
                for hb in range(0, nb, PZ):
                    nh = min(PZ, nb - hb)
                    nc.sync.dma_start(
                        x3[:, hb:hb + nh, :],
                        xw_d.ap()[b0 + hb:b0 + hb + nh, :, :]
                        .rearrange("b p w -> p b w"))
                    if ch == 0 and hb == 0:
                        # idx is first needed by the first scatter, well
                        # after the first xw piece — don't delay that piece
                        nc.sync.dma_start(idx_t[:], idx_d.ap()[:, :])
                    # two halving levels of the score row-sums in 2x-mode
                    # ops: h[p,t,j] = xw[p,t,j] + xw[p,t,64+j], then
                    # h2[p,t,j] = h[p,t,j] + h[p,t,32+j]; then one 1x
                    # tensor_reduce and one exp for the whole piece
                    t0, t1 = hb * TPB, (hb + nh) * TPB
                    nc.vector.tensor_tensor(
                        out=h4[:, t0:t1, :],
                        in0=x4[:, t0:t1, 0:64], in1=x4[:, t0:t1, 64:128],
                        op=Alu.add)
                    nc.vector.tensor_tensor(
                        out=h24[:, t0:t1, :],
                        in0=h4[:, t0:t1, 0:32], in1=h4[:, t0:t1, 32:64],
                        op=Alu.add)
                    nc.vector.tensor_reduce(
                        out=sc_c[:, t0:t1], in_=h3[:, t0:t1, :],
                        axis=Ax.X, op=Alu.add)
                    nc.scalar.activation(
                        out=zbuf[:, b0 * TPB + t0:b0 * TPB + t1],
                        in_=sc_c[:, t0:t1], func=Act.Exp)

                for bi in range(nb):
                    b = b0 + bi
                    me_t = mep.tile([P, W], bf16, tag="me")
                    nc.gpsimd.local_scatter(
                        out_ap=me_t[:],
                        data_ap=zbuf[:, b * TPB:(b + 1) * TPB],
                        idxs_ap=idx_t[:, b * TPB:(b + 1) * TPB],
                        channels=P, num_elems=W, num_idxs=TPB)

                    ps = psp.tile([P, P], f32, tag="acc")
                    for t in range(TPB):
                        nc.tensor.matmul(
                            ps[:],
                            lhsT=me_t[:, t * P:(t + 1) * P],
                            rhs=xw_c[:, (bi * TPB + t) * P:
                                     (bi * TPB + t + 1) * P],
                            start=(t == 0), stop=(t == TPB - 1))

                    nc.scalar.activation(
                        out=obuf[:, b * P:(b + 1) * P], in_=ps[:],
                        func=Act.Copy)
                done = b0 + nb
                if done - flushed >= (B + 3) // 4 or done == B:
                    pend = (flushed, done)
                    flushed = done
                if (done * 2 >= B and z_flushed == 0) or done == B:
                    pend_z = (z_flushed, done)
                    z_flushed = done

            issue_flush(pend, pend_z)

    nc.compile()
    return nc


# ---------------------------------------------------------------- host side
def _pack_blocks(counts):
    blocks = []
    s, nseg = 0, len(counts)
    while s < nseg:
        rows, s0 = 0, s
        while s < nseg and s - s0 < MAX_SEGS_PER_BLOCK:
            c = counts[s]
            if rows + c > ROWS_PER_BLOCK:
                break
            rows += int(c)
            s += 1
        assert s > s0, f"segment {s0} with {counts[s0]} rows exceeds a block"
        blocks.append((s0, s, rows))
    return blocks


def _numpy_fallback(x, labels, w, b):
    scores = x.astype(np.float64) @ w.astype(np.float64) + float(b)
    scores -= scores.max()
    e = np.exp(scores)
    a = e / e.sum()
    out = np.zeros((NUM_SEGMENTS, x.shape[1]), np.float64)
    np.add.at(out, labels, x * a[:, None])
    return out.astype(np.float32)


def kernel(x, monomer_labels_i, attn_w, attn_b):
    from concourse import bass_utils

    x = np.ascontiguousarray(np.asarray(x, dtype=np.float32))
    labels = np.asarray(monomer_labels_i).astype(np.int64)
    w = np.asarray(attn_w, dtype=np.float32)
    b = np.float32(np.asarray(attn_b))

    if np.abs(w).min() < 1e-30 or np.bincount(
            labels, minlength=NUM_SEGMENTS).max() > ROWS_PER_BLOCK:
        return _numpy_fallback(x, labels, w, b)

    order = np.argsort(labels, kind="stable")
    labels_s = labels[order]
    counts = np.bincount(labels, minlength=NUM_SEGMENTS)
    blocks = _pack_blocks(counts)
    nblocks = len(blocks)
    B = (nblocks + N_CORES - 1) // N_CORES
    NT = B * TPB
    seg_row_start = np.zeros(NUM_SEGMENTS + 1, np.int64)
    np.cumsum(counts, out=seg_row_start[1:])

    xw = x[order] * w[None, :]
    xw_hi = xw.astype(ml_dtypes.bfloat16)

    # per-tile one-hot column index: t*128 + rel_label (pad rows: -1)
    tile_base = (np.arange(ROWS_PER_BLOCK) // P).astype(np.int16) * P

    in_maps = []
    meta = []
    for c in range(N_CORES):
        xw_dev = np.zeros((B, P, TPB, P), ml_dtypes.bfloat16)
        idx_dev = np.full((B, TPB, P), -1, np.int16)
        meta_c = []
        for bi in range(B):
            gi = c * B + bi
            if gi >= nblocks:
                meta_c.append(None)
                continue
            s0, s1, rows = blocks[gi]
            r0 = seg_row_start[s0]

            full = np.zeros((ROWS_PER_BLOCK, D), ml_dtypes.bfloat16)
            full[:rows] = xw_hi[r0:r0 + rows]
            xw_dev[bi] = full.reshape(TPB, P, D).transpose(1, 0, 2)

            fi = np.full(ROWS_PER_BLOCK, -1, np.int16)
            fi[:rows] = (labels_s[r0:r0 + rows] - s0).astype(np.int16) + \
                tile_base[:rows]
            idx_dev[bi] = fi.reshape(TPB, P)
            meta_c.append((int(s0), int(s1)))
        meta.append(meta_c)
        # idx layout on device: [P, B*TPB], column b*TPB+t
        in_maps.append({"xw": xw_dev.reshape(B, P, TPB * P),
                        "idx": np.ascontiguousarray(
                            idx_dev.reshape(NT, P).T)})

    if B not in _COMPILED:
        _COMPILED[B] = _build_kernel(B)
    nc = _COMPILED[B]

    res = bass_utils.run_bass_kernel_spmd(nc, in_maps,
                                          core_ids=list(range(N_CORES)))

    # ---- gather / unshard
    out = np.zeros((NUM_SEGMENTS, D), np.float32)
    Z = 0.0
    for c in range(N_CORES):
        r = res.results[c]
        Z += float(r["zpart"].astype(np.float64).sum())
        out_dev = r["out"]
        for bi in range(B):
            m = meta[c][bi]
            if m is None:
                continue
            s0, s1 = m
            out[s0:s1] = out_dev[bi, :s1 - s0, :].astype(np.float32)
    # pad rows have xw == 0 -> score 0 -> e = exp(0) = 1 each
    n_pad_rows = N_CORES * B * ROWS_PER_BLOCK - N
    Z -= float(n_pad_rows)
    out /= (w[None, :] * np.float32(Z))
    return out.astype(np.float32)


if __name__ == "__main__":
    from ref_io import get
    inputs, expected = get()
    out = kernel(**inputs)
    err = np.abs(out - expected)
    print("absmax err:", err.max(), "scale-rel:",
          err.max() / np.abs(expected).max())
